# revision 1
# baseline (speedup 1.0000x reference)
"""ChebNet (K=3, 2 layers) node classification on 8 Trainium2 NeuronCores.

Node-sharded (12500 nodes/core, padded to 12544 = 98*128 slots), slots
degree-sorted per core so each ELL propagation round covers a prefix of
slot-blocks. Each (round, block) is one 128-row indirect-DMA gather from
the AllGather'd table, accumulating into SBUF via the DMA CCE add
(round 0 writes with bypass over all 98 blocks, so no memset).
Chebyshev recurrence/scaling is folded into per-slot dinv scalings done
as single whole-tile DVE ops with [P,B,1]->[P,B,F] broadcast APs:
  Tx1 = -dinv * u(xtil),   xtil = dinv * x
  Tx2 = -2*dinv*u(Ttil1) - Tx0,  Ttil1 = dinv * Tx1
Layers run as For_i hardware loops over 49 block-pairs: stage copy ->
PE pair-transpose -> W-stationary matmuls (transposed output [HID,nodes])
-> per-partition-bias activation -> PE transpose back. Stable log_softmax
uses batched strided APs. Weights replicated across cores.
"""

import numpy as np

import concourse.bass as bass
from concourse.bass import ds
import concourse.mybir as mybir
import concourse.tile as tile
from concourse import bass_utils
from concourse.masks import make_identity

NCORES = 8
P = 128
N = 100000
E = 1600000
F = 64
HID = 64
C = 16
NPC = 12500          # nodes per core
BLOCKS = 98          # ceil(12544/128)
SLOTS = BLOCKS * P   # 12544 padded slots per core
GTOT = NCORES * SLOTS        # 100352
ZROW = GTOT                  # index of zero row in gathered tensors
GFULL = GTOT + P             # gather source rows incl. zero rows


def _cap_waits(nc):
    """Walrus accepts at most 1 folded sem-wait per non-EVSEM instruction."""
    for bb in nc.main_func.blocks:
        new_insts = []
        for inst in bb.instructions:
            si = inst.sync_info
            waits = list(si.on_wait) if si is not None and si.on_wait else []
            cap = 2 if isinstance(inst, mybir.InstEventSemaphore) else 1
            if len(waits) > cap:
                excess, keep = waits[:-cap], waits[-cap:]
                while excess:
                    chunk, excess = excess[:2], excess[2:]
                    ev = mybir.InstEventSemaphore(
                        name=f"I-{nc.next_id()}",
                        ins=[],
                        outs=[],
                        engine=inst.engine,
                        sync_info=mybir.SyncInfo(on_wait=chunk, on_update=[]),
                    )
                    new_insts.append(ev)
                si.on_wait = keep
            new_insts.append(inst)
        bb.instructions[:] = new_insts


def _prep_meta(edge_index):
    """Fast phase: degrees, slot assignment, round shapes (T/offs)."""
    row = edge_index[0]
    deg = np.bincount(row, minlength=N)
    dinv = np.where(
        deg > 0, 1.0 / np.sqrt(np.maximum(deg, 1)), 0.0
    ).astype(np.float32)

    # per-core degree-sorted slot assignment (slot s -> partition s%P, block s//P)
    d2 = deg.reshape(NCORES, NPC)
    order = np.argsort(-d2, axis=1, kind="stable")
    slot_node = order + (np.arange(NCORES) * NPC)[:, None]  # [NCORES, NPC]
    deg_sorted = np.take_along_axis(d2, order, axis=1)

    s = np.arange(NPC)
    sp, sb = s % P, s // P
    gid = np.empty(N, np.int64)
    gid[slot_node] = (np.arange(NCORES) * SLOTS)[:, None] + sp * BLOCKS + sb
    slot_of = np.empty(N, np.int64)
    slot_of[slot_node] = s[None, :]

    maxdeg = int(deg.max())
    cnt = np.zeros((NCORES, maxdeg + 1), np.int64)
    for c in range(NCORES):
        cnt[c] = np.bincount(deg_sorted[c], minlength=maxdeg + 1)
    n_act = NPC - np.cumsum(cnt, axis=1)       # [NCORES, maxdeg+1]: # deg > j
    act_max = n_act.max(axis=0)[:maxdeg]
    T = -(-act_max // P)                        # ceil / 128
    T[0] = BLOCKS                               # full coverage for bypass round
    offs = np.concatenate([[0], np.cumsum(T)]).astype(np.int64)
    tot_cols = int(offs[-1])
    return deg, dinv, slot_node, gid, slot_of, T, offs, tot_cols


def _prep_fill(x, edge_index, deg, dinv, slot_node, gid, slot_of, offs, tot_cols):
    """Slow phase: edge sort, ELL index fill, blocked tensors. Runs in a
    background thread overlapped with _build (numpy releases the GIL)."""
    row = edge_index[0]
    col = edge_index[1]
    # edges sorted by destination; rank = j-th in-edge of its destination
    eorder = np.argsort(row, kind="stable")
    srt_row = row[eorder].astype(np.int64)
    srt_col = col[eorder]
    starts = np.concatenate([[0], np.cumsum(deg)[:-1]])
    rank = np.arange(E) - starts[srt_row]

    # ELL: idx[c, p, offs[j]+b] = gid of source of slot (p,b)'s j-th edge
    idx = np.full((NCORES, P, tot_cols), ZROW, np.int32)
    e_core = srt_row // NPC
    e_slot = slot_of[srt_row]
    idx[e_core, e_slot % P, offs[rank] + e_slot // P] = gid[srt_col]

    # blocked per-core tensors [P, BLOCKS*f]
    xr = np.zeros((NCORES, SLOTS, F), np.float32)
    xr[:, :NPC] = x[slot_node]
    xb = xr.reshape(NCORES, BLOCKS, P, F).transpose(0, 2, 1, 3).reshape(
        NCORES, P, BLOCKS * F
    )
    dr = np.zeros((NCORES, SLOTS), np.float32)
    dr[:, :NPC] = dinv[slot_node]
    dinvb = dr.reshape(NCORES, BLOCKS, P).transpose(0, 2, 1).copy()
    return idx, xb, dinvb


def _prep(x, edge_index):
    """Host-side graph preprocessing: sharding, degree sort, ELL rounds."""
    deg, dinv, slot_node, gid, slot_of, T, offs, tot_cols = _prep_meta(edge_index)
    idx, xb, dinvb = _prep_fill(
        x, edge_index, deg, dinv, slot_node, gid, slot_of, offs, tot_cols
    )
    return idx, xb, dinvb, slot_node, [int(t) for t in T], offs, tot_cols


def _build(T, offs, tot_cols):
    nc = bass.Bass(trn_type="TRN2", num_devices=NCORES, debug=False)
    dt = mybir.dt
    f32 = dt.float32
    x_in = nc.dram_tensor("x_in", [P, BLOCKS * F], f32, kind="ExternalInput")
    dinv_in = nc.dram_tensor("dinv_in", [P, BLOCKS], f32, kind="ExternalInput")
    idx_in = nc.dram_tensor("idx_in", [P, tot_cols], dt.int32, kind="ExternalInput")
    w1_in = nc.dram_tensor("w1_in", [3, F, HID], f32, kind="ExternalInput")
    b1_in = nc.dram_tensor("b1_in", [1, HID], f32, kind="ExternalInput")
    w2_in = nc.dram_tensor("w2_in", [3, HID, C], f32, kind="ExternalInput")
    b2_in = nc.dram_tensor("b2_in", [1, C], f32, kind="ExternalInput")
    o_out = nc.dram_tensor("o_out", [P, BLOCKS * C], f32, kind="ExternalOutput")

    nrounds = len(T)
    BF = BLOCKS * F

    with tile.TileContext(nc) as tc:
        with (
            tc.tile_pool(name="sb", bufs=1) as sb,
            tc.tile_pool(name="ps", bufs=1, space="PSUM") as ps,
            tc.tile_pool(name="dram", bufs=1, space="DRAM") as dram,
        ):
            # ---- loads ----
            idx_sb = sb.tile([P, tot_cols], dt.int32)
            nc.gpsimd.dma_start(idx_sb[:], idx_in.ap())
            x_sb = sb.tile([P, BF], f32)
            nc.sync.dma_start(x_sb[:], x_in.ap())
            dinv_sb = sb.tile([P, BLOCKS], f32)
            nc.sync.dma_start(dinv_sb[:], dinv_in.ap())
            w1_sb = sb.tile([P, 3 * HID], f32)
            w2_sb = sb.tile([P, 3 * C], f32)
            for hb in (0, F):
                nc.sync.dma_start(
                    w1_sb[hb : hb + F, :].rearrange("f (k h) -> f k h", k=3),
                    w1_in.ap().rearrange("k f h -> f k h"),
                )
                nc.sync.dma_start(
                    w2_sb[hb : hb + F, :].rearrange("f (k h) -> f k h", k=3),
                    w2_in.ap().rearrange("k f h -> f k h"),
                )
            b1_sb = sb.tile([HID, 1], f32)
            nc.sync.dma_start(b1_sb[:], b1_in.ap().rearrange("o h -> h o"))
            b2_sb = sb.tile([C, 1], f32)
            nc.sync.dma_start(b2_sb[:], b2_in.ap().rearrange("o c -> c o"))
            ident = sb.tile([P, P], f32)
            make_identity(nc, ident[:])
            zero_sb = sb.tile([P, F], f32)
            nc.vector.memset(zero_sb[:], 0.0)
            absorb_sb = sb.tile([1, F], f32)

            # derived per-slot scalings [P, BLOCKS]
            ndinv = sb.tile([P, BLOCKS], f32)   # -dinv
            nc.vector.tensor_scalar_mul(ndinv[:], dinv_sb[:], -1.0)
            ndinv2 = sb.tile([P, BLOCKS], f32)  # -dinv^2
            nc.vector.tensor_tensor(
                out=ndinv2[:], in0=ndinv[:], in1=dinv_sb[:], op=mybir.AluOpType.mult
            )
            n2dinv = sb.tile([P, BLOCKS], f32)  # -2*dinv
            nc.vector.tensor_scalar_mul(n2dinv[:], dinv_sb[:], -2.0)

            # working tensors
            u_sb = sb.tile([P, BF], f32)     # gather accumulator
            pub_sb = sb.tile([P, BF], f32)   # scaled tensor to publish
            tx1 = sb.tile([P, BF], f32)
            tx2 = sb.tile([P, BF], f32)
            h_sb = sb.tile([P, BF], f32)
            o_sb = sb.tile([P, BF], f32)     # layer-2 out, C cols per 64-block

            # dram tensors for collectives; zero rows written once
            agin = [dram.tile([SLOTS, F], f32, name=f"agin{i}") for i in range(4)]
            full = [dram.tile([GFULL, F], f32, name=f"full{i}") for i in range(4)]
            for i in range(4):
                nc.sync.dma_start(full[i][GTOT : GTOT + P, :], zero_sb[:])

            def bscale(dst, src, sc):
                """dst[p, b*F+f] = src[p, b*F+f] * sc[p, b] — one DVE op."""
                nc.vector.tensor_tensor(
                    out=dst[:].rearrange("p (b f) -> p b f", f=F),
                    in0=src[:].rearrange("p (b f) -> p b f", f=F),
                    in1=sc[:].unsqueeze(2).to_broadcast([P, BLOCKS, F]),
                    op=mybir.AluOpType.mult,
                )

            def publish(i):
                # pub_sb [P, BLOCKS*F] -> agin rows (p*BLOCKS+b) -> allgather
                nc.sync.dma_start(
                    agin[i][:].rearrange("(p b) f -> p (b f)", p=P), pub_sb[:]
                )
                nc.gpsimd.collective_compute(
                    "AllGather",
                    mybir.AluOpType.bypass,
                    replica_groups=[list(range(NCORES))],
                    ins=[agin[i].opt()],
                    outs=[full[i][0:GTOT, :].opt()],
                )
                # absorb the collective wait on Pool before gathers
                nc.gpsimd.dma_start(absorb_sb[0:1, 0:F], full[i][0:1, :])

            def prop(i):
                # one [128,1]-offset gather per (round, block); round 0 covers
                # all 98 blocks with bypass (no memset needed)
                for j in range(nrounds):
                    for b in range(T[j]):
                        cidx = int(offs[j]) + b
                        nc.gpsimd.indirect_dma_start(
                            out=u_sb[:, b * F : (b + 1) * F],
                            out_offset=None,
                            in_=full[i][:],
                            in_offset=bass.IndirectOffsetOnAxis(
                                ap=idx_sb[:, cidx : cidx + 1], axis=0
                            ),
                            compute_op=(
                                mybir.AluOpType.bypass
                                if j == 0
                                else mybir.AluOpType.add
                            ),
                        )

            # staging tiles for the layer loops
            st0 = sb.tile([P, P], f32, name="st0")
            st1 = sb.tile([P, P], f32, name="st1")
            st2 = sb.tile([P, P], f32, name="st2")
            t0 = sb.tile([P, P], f32, name="t0")
            t1 = sb.tile([P, P], f32, name="t1")
            t2 = sb.tile([P, P], f32, name="t2")
            oTs = sb.tile([HID, P], f32, name="oTs")

            def layer(in0, in1, in2, w_sb, b_sb, hid, out_sb, act):
                with tc.For_i(0, BF, 2 * F) as i:
                    nc.vector.tensor_copy(st0[:], in0[:, ds(i, 2 * F)])
                    nc.vector.tensor_copy(st1[:], in1[:, ds(i, 2 * F)])
                    nc.vector.tensor_copy(st2[:], in2[:, ds(i, 2 * F)])
                    p0 = ps.tile([P, P], f32, tag="p0")
                    p1 = ps.tile([P, P], f32, tag="p1")
                    p2 = ps.tile([P, P], f32, tag="p2")
                    nc.tensor.transpose(out=p0[:], in_=st0[:], identity=ident[:])
                    nc.tensor.transpose(out=p1[:], in_=st1[:], identity=ident[:])
                    nc.tensor.transpose(out=p2[:], in_=st2[:], identity=ident[:])
                    nc.vector.tensor_copy(t0[:], p0[:])
                    nc.vector.tensor_copy(t1[:], p1[:])
                    nc.vector.tensor_copy(t2[:], p2[:])
                    for half in (0, F):
                        op = ps.tile([hid, P], f32, tag=f"op{half}")
                        nc.tensor.matmul(
                            op[:], lhsT=w_sb[half : half + F, 0:hid],
                            rhs=t0[half : half + F, :], start=True, stop=False,
                        )
                        nc.tensor.matmul(
                            op[:], lhsT=w_sb[half : half + F, hid : 2 * hid],
                            rhs=t1[half : half + F, :], start=False, stop=False,
                        )
                        nc.tensor.matmul(
                            op[:], lhsT=w_sb[half : half + F, 2 * hid : 3 * hid],
                            rhs=t2[half : half + F, :], start=False, stop=True,
                        )
                        nc.scalar.activation(
                            oTs[0:hid, :], op[:], act, bias=b_sb[:, 0:1], scale=1.0
                        )
                        ph = ps.tile([P, hid], f32, tag=f"ph{half}")
                        nc.tensor.transpose(
                            out=ph[:], in_=oTs[0:hid, :], identity=ident[0:hid, 0:hid]
                        )
                        nc.vector.tensor_copy(
                            out_sb[:, ds(i + half, hid)], ph[:]
                        )

            # ---- layer 1 ----
            bscale(pub_sb, x_sb, dinv_sb)       # xtil = dinv*x
            publish(0)
            prop(0)                              # u_sb = u1
            bscale(tx1, u_sb, ndinv)             # Tx1 = -dinv*u1
            bscale(pub_sb, u_sb, ndinv2)         # Ttil1 = dinv*Tx1
            publish(1)
            prop(1)                              # u_sb = u2
            bscale(tx2, u_sb, n2dinv)            # -2dinv*u2
            nc.vector.tensor_tensor(
                out=tx2[:], in0=tx2[:], in1=x_sb[:], op=mybir.AluOpType.subtract
            )                                    # Tx2 = -2dinv*u2 - Tx0
            layer(x_sb, tx1, tx2, w1_sb, b1_sb, HID, h_sb,
                  mybir.ActivationFunctionType.Relu)

            # ---- layer 2 ----
            bscale(pub_sb, h_sb, dinv_sb)        # htil
            publish(2)
            prop(2)                              # u_sb = u3
            bscale(tx1, u_sb, ndinv)             # Tx1'
            bscale(pub_sb, u_sb, ndinv2)         # Ttil1'
            publish(3)
            prop(3)                              # u_sb = u4
            bscale(tx2, u_sb, n2dinv)
            nc.vector.tensor_tensor(
                out=tx2[:], in0=tx2[:], in1=h_sb[:], op=mybir.AluOpType.subtract
            )                                    # Tx2'
            layer(h_sb, tx1, tx2, w2_sb, b2_sb, C, o_sb,
                  mybir.ActivationFunctionType.Identity)

            # ---- stable log_softmax over C cols of each 64-block ----
            ov = o_sb[:].rearrange("p (b f) -> p b f", f=F)[:, :, 0:C]
            mx = sb.tile([P, BLOCKS], f32)
            nc.vector.tensor_reduce(
                out=mx[:].unsqueeze(2), in_=ov,
                op=mybir.AluOpType.max, axis=mybir.AxisListType.X,
            )
            sh_sb = sb.tile([P, BLOCKS * C], f32)
            shv = sh_sb[:].rearrange("p (b c) -> p b c", c=C)
            nc.vector.tensor_tensor(
                out=shv, in0=ov,
                in1=mx[:].unsqueeze(2).to_broadcast([P, BLOCKS, C]),
                op=mybir.AluOpType.subtract,
            )
            e_sb = sb.tile([P, BLOCKS * C], f32)
            nc.scalar.activation(
                e_sb[:], sh_sb[:], mybir.ActivationFunctionType.Exp
            )
            ssum = sb.tile([P, BLOCKS], f32)
            nc.vector.tensor_reduce(
                out=ssum[:].unsqueeze(2),
                in_=e_sb[:].rearrange("p (b c) -> p b c", c=C),
                op=mybir.AluOpType.add, axis=mybir.AxisListType.X,
            )
            lns = sb.tile([P, BLOCKS], f32)
            nc.scalar.activation(lns[:], ssum[:], mybir.ActivationFunctionType.Ln)
            sm_sb = sb.tile([P, BLOCKS * C], f32)
            nc.vector.tensor_tensor(
                out=sm_sb[:].rearrange("p (b c) -> p b c", c=C),
                in0=shv,
                in1=lns[:].unsqueeze(2).to_broadcast([P, BLOCKS, C]),
                op=mybir.AluOpType.subtract,
            )
            nc.sync.dma_start(o_out.ap(), sm_sb[:])

    _cap_waits(nc)
    return nc


def kernel(x, edge_index, W1, b1, W2, b2):
    x = np.asarray(x, np.float32)
    edge_index = np.asarray(edge_index, np.int32)
    W1 = np.asarray(W1, np.float32)
    b1 = np.asarray(b1, np.float32)
    W2 = np.asarray(W2, np.float32)
    b2 = np.asarray(b2, np.float32)

    # overlap the heavy prep (edge sort + ELL fill, GIL-releasing numpy) and
    # the axon/PJRT backend bring-up with the bass build
    import threading

    def _warm_backend():
        try:
            import jax

            jax.devices()
        except Exception:
            pass

    bk = threading.Thread(target=_warm_backend, daemon=True)
    bk.start()

    deg, dinv, slot_node, gid, slot_of, T, offs, tot_cols = _prep_meta(edge_index)
    T = [int(t) for t in T]

    fill_out = {}

    def _fill():
        fill_out["r"] = _prep_fill(
            x, edge_index, deg, dinv, slot_node, gid, slot_of, offs, tot_cols
        )

    th = threading.Thread(target=_fill)
    th.start()
    nc = _build(T, offs, tot_cols)
    th.join()
    idx, xb, dinvb = fill_out["r"]

    in_maps = []
    for c in range(NCORES):
        in_maps.append(
            {
                "x_in": xb[c],
                "dinv_in": dinvb[c],
                "idx_in": idx[c],
                "w1_in": W1,
                "b1_in": b1.reshape(1, HID),
                "w2_in": W2,
                "b2_in": b2.reshape(1, C),
            }
        )
    res = bass_utils.run_bass_kernel_spmd(nc, in_maps, core_ids=list(range(NCORES)))

    out = np.empty((N, C), np.float32)
    for c in range(NCORES):
        ob = res.results[c]["o_out"]  # [P, BLOCKS*C]
        rows = ob.reshape(P, BLOCKS, C).transpose(1, 0, 2).reshape(SLOTS, C)
        out[slot_node[c]] = rows[:NPC]
    return out



# revision 2
# speedup vs baseline: 2.9314x; 2.9314x over previous
"""ChebNet (K=3, 2 layers) node classification on 8 Trainium2 NeuronCores.

Node-sharded (12500 nodes/core, padded to 12544 = 98*128 slots), slots
degree-sorted per core so ELL propagation rounds cover a prefix of
slot-blocks. Gathers run as For_i hardware loops over block classes
(blocks grouped by padded round-count R): each iteration copies that
block's R offset columns into a fixed staging tile (indirect-DMA offsets
must be physical APs), then issues R 128-row indirect gathers from the
AllGather'd table into u_sb via DMA CCE (r=0 bypass initializes, r>0
adds). Chebyshev recurrence/scaling folded into per-slot dinv scalings
as whole-tile DVE ops with [P,B,1]->[P,B,F] broadcast APs.
Layers run as For_i loops over 49 block-pairs: stage copy -> PE
pair-transpose -> W-stationary matmuls -> per-partition-bias activation
-> PE transpose back (layer-2 output packed to C=16-col pitch).
x ships host->device as fp16 (cast to f32 on device); log-softmax output
ships back fp16. Build uses Bacc (rust lowering); the runner AOT-compiles
while a background thread preps inputs and device_puts them sharded, so
H2D transfer overlaps NEFF compile.
"""

import threading

import numpy as np

import concourse.bass as bass
from concourse.bacc import Bacc
from concourse.bass import ds
import concourse.mybir as mybir
import concourse.tile as tile
from concourse import bass2jax as b2j
from concourse.masks import make_identity

NCORES = 8
P = 128
N = 100000
E = 1600000
F = 64
HID = 64
C = 16
NPC = 12500          # nodes per core
BLOCKS = 98          # ceil(12544/128)
SLOTS = BLOCKS * P   # 12544 padded slots per core
GTOT = NCORES * SLOTS        # 100352
ZROW = GTOT                  # index of zero row in gathered tensors
GFULL = GTOT + P             # gather source rows incl. zero rows
KCLS = 6                     # max gather block-classes per prop
XCH = 8                      # x fp16 load chunks


def _cap_waits(nc):
    """Walrus accepts at most 1 folded sem-wait per non-EVSEM instruction."""
    for bb in nc.main_func.blocks:
        new_insts = []
        for inst in bb.instructions:
            si = inst.sync_info
            waits = list(si.on_wait) if si is not None and si.on_wait else []
            cap = 2 if isinstance(inst, mybir.InstEventSemaphore) else 1
            if len(waits) > cap:
                excess, keep = waits[:-cap], waits[-cap:]
                while excess:
                    chunk, excess = excess[:2], excess[2:]
                    ev = mybir.InstEventSemaphore(
                        name=f"I-{nc.next_id()}",
                        ins=[],
                        outs=[],
                        engine=inst.engine,
                        sync_info=mybir.SyncInfo(on_wait=chunk, on_update=[]),
                    )
                    new_insts.append(ev)
                si.on_wait = keep
            new_insts.append(inst)
        bb.instructions[:] = new_insts


def _classes_of(T):
    """Partition blocks 0..97 into <=KCLS classes of padded round-count R.

    R_b = #{j: T[j] > b} is non-increasing in b; DP minimizes total padded
    gather area sum(R[b0] * len)."""
    T = np.asarray(T)
    Rb = (T[None, :] > np.arange(BLOCKS)[:, None]).sum(axis=1)  # [BLOCKS]
    INF = 1 << 60
    dp = np.full((KCLS + 1, BLOCKS + 1), INF, np.int64)
    nxt = np.zeros((KCLS + 1, BLOCKS + 1), np.int64)
    dp[:, BLOCKS] = 0
    for k in range(1, KCLS + 1):
        for b in range(BLOCKS - 1, -1, -1):
            lens = np.arange(1, BLOCKS - b + 1)
            costs = Rb[b] * lens + dp[k - 1, b + 1 : BLOCKS + 1]
            e = int(np.argmin(costs))
            dp[k, b] = costs[e]
            nxt[k, b] = b + 1 + e
    classes = []
    b, k = 0, KCLS
    while b < BLOCKS:
        e = int(nxt[k, b])
        classes.append((b, e, int(Rb[b])))
        b = e
        k -= 1
    colstart = np.zeros(BLOCKS, np.int64)
    base = 0
    cls = []
    for b0, e, R in classes:
        colstart[b0:e] = base + (np.arange(e - b0)) * R
        cls.append((b0, e, R, base))
        base += (e - b0) * R
    return cls, colstart, int(base)


def _prep_meta(edge_index):
    """Fast phase: degrees, slot assignment, gather classes."""
    row = edge_index[0]
    deg = np.bincount(row, minlength=N)
    dinv = np.where(
        deg > 0, 1.0 / np.sqrt(np.maximum(deg, 1)), 0.0
    ).astype(np.float32)

    # per-core degree-sorted slot assignment (slot s -> partition s%P, block s//P)
    d2 = deg.reshape(NCORES, NPC)
    order = np.argsort(-d2, axis=1, kind="stable")
    slot_node = order + (np.arange(NCORES) * NPC)[:, None]  # [NCORES, NPC]
    deg_sorted = np.take_along_axis(d2, order, axis=1)

    s = np.arange(NPC)
    sp, sb = s % P, s // P
    gid = np.empty(N, np.int64)
    gid[slot_node] = (np.arange(NCORES) * SLOTS)[:, None] + sp * BLOCKS + sb
    slot_of = np.empty(N, np.int64)
    slot_of[slot_node] = s[None, :]

    maxdeg = int(deg.max())
    cnt = np.zeros((NCORES, maxdeg + 1), np.int64)
    for c in range(NCORES):
        cnt[c] = np.bincount(deg_sorted[c], minlength=maxdeg + 1)
    n_act = NPC - np.cumsum(cnt, axis=1)       # [NCORES, maxdeg+1]: # deg > j
    act_max = n_act.max(axis=0)[:maxdeg]
    T = -(-act_max // P)                        # ceil / 128
    T[0] = BLOCKS                               # full coverage for init round
    classes, colstart, tot_cols = _classes_of(T)
    return deg, dinv, slot_node, gid, slot_of, classes, colstart, tot_cols


def _prep_fill(x, edge_index, deg, dinv, slot_node, gid, slot_of, colstart, tot_cols):
    """Slow phase: edge sort, block-major ELL fill, blocked tensors."""
    row = edge_index[0]
    col = edge_index[1]
    eorder = np.argsort(row, kind="stable")
    srt_row = row[eorder].astype(np.int64)
    srt_col = col[eorder]
    starts = np.concatenate([[0], np.cumsum(deg)[:-1]])
    rank = np.arange(E) - starts[srt_row]

    # block-major ELL: idx[c, p, colstart[b] + r] = gid of source of the
    # r-th edge of slot (p, b)
    idx = np.full((NCORES, P, tot_cols), ZROW, np.int32)
    e_core = srt_row // NPC
    e_slot = slot_of[srt_row]
    idx[e_core, e_slot % P, colstart[e_slot // P] + rank] = gid[srt_col]

    # blocked per-core tensors [P, BLOCKS*f]; x ships fp16
    xr = np.zeros((NCORES, SLOTS, F), np.float16)
    xr[:, :NPC] = x[slot_node]
    xb = xr.reshape(NCORES, BLOCKS, P, F).transpose(0, 2, 1, 3).reshape(
        NCORES, P, BLOCKS * F
    )
    dr = np.zeros((NCORES, SLOTS), np.float32)
    dr[:, :NPC] = dinv[slot_node]
    dinvb = dr.reshape(NCORES, BLOCKS, P).transpose(0, 2, 1).copy()
    return idx, xb, dinvb


def _build(classes, tot_cols):
    nc = Bacc(trn_type="TRN2", num_devices=NCORES, debug=False)
    dt = mybir.dt
    f32 = dt.float32
    f16 = dt.float16
    x_in = nc.dram_tensor("x_in", [P, BLOCKS * F], f16, kind="ExternalInput")
    dinv_in = nc.dram_tensor("dinv_in", [P, BLOCKS], f32, kind="ExternalInput")
    idx_in = nc.dram_tensor("idx_in", [P, tot_cols], dt.int32, kind="ExternalInput")
    w1_in = nc.dram_tensor("w1_in", [3, F, HID], f32, kind="ExternalInput")
    b1_in = nc.dram_tensor("b1_in", [1, HID], f32, kind="ExternalInput")
    w2_in = nc.dram_tensor("w2_in", [3, HID, C], f32, kind="ExternalInput")
    b2_in = nc.dram_tensor("b2_in", [1, C], f32, kind="ExternalInput")
    o_out = nc.dram_tensor("o_out", [P, BLOCKS * C], f16, kind="ExternalOutput")

    BF = BLOCKS * F
    XC = BF // XCH  # fp16 x chunk cols
    RMAX = max(r for _, _, r, _ in classes)

    with tile.TileContext(nc) as tc:
        with (
            tc.tile_pool(name="sb", bufs=1) as sb,
            tc.tile_pool(name="ps", bufs=1, space="PSUM") as ps,
            tc.tile_pool(name="dram", bufs=1, space="DRAM") as dram,
        ):
            # ---- loads ----
            idx_sb = sb.tile([P, tot_cols], dt.int32)
            nc.gpsimd.dma_start(idx_sb[:], idx_in.ap())
            x_sb = sb.tile([P, BF], f32)
            x16a = sb.tile([P, XC], f16)
            x16b = sb.tile([P, XC], f16)
            for ch in range(XCH):
                stg = x16a if ch % 2 == 0 else x16b
                nc.sync.dma_start(stg[:], x_in.ap()[:, ch * XC : (ch + 1) * XC])
                nc.vector.tensor_copy(x_sb[:, ch * XC : (ch + 1) * XC], stg[:])
            dinv_sb = sb.tile([P, BLOCKS], f32)
            nc.sync.dma_start(dinv_sb[:], dinv_in.ap())
            w1_sb = sb.tile([P, 3 * HID], f32)
            w2_sb = sb.tile([P, 3 * C], f32)
            for hb in (0, F):
                nc.sync.dma_start(
                    w1_sb[hb : hb + F, :].rearrange("f (k h) -> f k h", k=3),
                    w1_in.ap().rearrange("k f h -> f k h"),
                )
                nc.sync.dma_start(
                    w2_sb[hb : hb + F, :].rearrange("f (k h) -> f k h", k=3),
                    w2_in.ap().rearrange("k f h -> f k h"),
                )
            b1_sb = sb.tile([HID, 1], f32)
            nc.sync.dma_start(b1_sb[:], b1_in.ap().rearrange("o h -> h o"))
            b2_sb = sb.tile([C, 1], f32)
            nc.sync.dma_start(b2_sb[:], b2_in.ap().rearrange("o c -> c o"))
            ident = sb.tile([P, P], f32)
            make_identity(nc, ident[:])
            zero_sb = sb.tile([P, F], f32)
            nc.vector.memset(zero_sb[:], 0.0)
            absorb_sb = sb.tile([1, F], f32)

            # derived per-slot scalings [P, BLOCKS]
            ndinv = sb.tile([P, BLOCKS], f32)   # -dinv
            nc.vector.tensor_scalar_mul(ndinv[:], dinv_sb[:], -1.0)
            ndinv2 = sb.tile([P, BLOCKS], f32)  # -dinv^2
            nc.vector.tensor_tensor(
                out=ndinv2[:], in0=ndinv[:], in1=dinv_sb[:], op=mybir.AluOpType.mult
            )
            n2dinv = sb.tile([P, BLOCKS], f32)  # -2*dinv
            nc.vector.tensor_scalar_mul(n2dinv[:], dinv_sb[:], -2.0)

            # working tensors
            u_sb = sb.tile([P, BF], f32)     # gather accumulator
            pub_sb = sb.tile([P, BF], f32)   # scaled tensor to publish
            tx1 = sb.tile([P, BF], f32)
            tx2 = sb.tile([P, BF], f32)
            h_sb = sb.tile([P, BF], f32)
            o_sb = sb.tile([P, BLOCKS * C], f32)  # layer-2 out, packed C pitch
            stage = sb.tile([P, RMAX], dt.int32)  # physical gather offsets

            # dram tensors for collectives; zero rows written once
            agin = [dram.tile([SLOTS, F], f32, name=f"agin{i}") for i in range(4)]
            full = [dram.tile([GFULL, F], f32, name=f"full{i}") for i in range(4)]
            for i in range(4):
                nc.sync.dma_start(full[i][GTOT : GTOT + P, :], zero_sb[:])

            def bscale(dst, src, sc):
                """dst[p, b*F+f] = src[p, b*F+f] * sc[p, b] — one DVE op."""
                nc.vector.tensor_tensor(
                    out=dst[:].rearrange("p (b f) -> p b f", f=F),
                    in0=src[:].rearrange("p (b f) -> p b f", f=F),
                    in1=sc[:].unsqueeze(2).to_broadcast([P, BLOCKS, F]),
                    op=mybir.AluOpType.mult,
                )

            def publish(i):
                # pub_sb [P, BLOCKS*F] -> agin rows (p*BLOCKS+b) -> allgather
                nc.sync.dma_start(
                    agin[i][:].rearrange("(p b) f -> p (b f)", p=P), pub_sb[:]
                )
                nc.gpsimd.collective_compute(
                    "AllGather",
                    mybir.AluOpType.bypass,
                    replica_groups=[list(range(NCORES))],
                    ins=[agin[i].opt()],
                    outs=[full[i][0:GTOT, :].opt()],
                )
                # absorb the collective wait on Pool before gathers
                nc.gpsimd.dma_start(absorb_sb[0:1, 0:F], full[i][0:1, :])

            def gath(src, off_ap, dst_ap, r):
                nc.gpsimd.indirect_dma_start(
                    out=dst_ap,
                    out_offset=None,
                    in_=src[:],
                    in_offset=bass.IndirectOffsetOnAxis(ap=off_ap, axis=0),
                    compute_op=(
                        mybir.AluOpType.bypass if r == 0 else mybir.AluOpType.add
                    ),
                )

            def prop(i):
                # r=0 gather initializes each block (bypass); r>0 accumulate.
                for b0, b1, R, base in classes:
                    if b1 - b0 <= 2:
                        for b in range(b0, b1):
                            cs = base + (b - b0) * R
                            for r in range(R):
                                gath(
                                    full[i],
                                    idx_sb[:, cs + r : cs + r + 1],
                                    u_sb[:, b * F : (b + 1) * F],
                                    r,
                                )
                    else:
                        with tc.For_i(0, b1 - b0) as iv:
                            nc.vector.tensor_copy(
                                stage[:, 0:R], idx_sb[:, ds(iv * R + base, R)]
                            )
                            for r in range(R):
                                gath(
                                    full[i],
                                    stage[:, r : r + 1],
                                    u_sb[:, ds(iv * F + b0 * F, F)],
                                    r,
                                )

            # staging tiles for the layer loops
            st0 = sb.tile([P, P], f32, name="st0")
            st1 = sb.tile([P, P], f32, name="st1")
            st2 = sb.tile([P, P], f32, name="st2")
            t0 = sb.tile([P, P], f32, name="t0")
            t1 = sb.tile([P, P], f32, name="t1")
            t2 = sb.tile([P, P], f32, name="t2")
            oTs = sb.tile([HID, P], f32, name="oTs")

            def layer(in0, in1, in2, w_sb, b_sb, hid, out_sb, act):
                with tc.For_i(0, BF // (2 * F)) as k:
                    nc.vector.tensor_copy(st0[:], in0[:, ds(k * (2 * F), 2 * F)])
                    nc.vector.tensor_copy(st1[:], in1[:, ds(k * (2 * F), 2 * F)])
                    nc.vector.tensor_copy(st2[:], in2[:, ds(k * (2 * F), 2 * F)])
                    p0 = ps.tile([P, P], f32, tag="p0")
                    p1 = ps.tile([P, P], f32, tag="p1")
                    p2 = ps.tile([P, P], f32, tag="p2")
                    nc.tensor.transpose(out=p0[:], in_=st0[:], identity=ident[:])
                    nc.tensor.transpose(out=p1[:], in_=st1[:], identity=ident[:])
                    nc.tensor.transpose(out=p2[:], in_=st2[:], identity=ident[:])
                    nc.vector.tensor_copy(t0[:], p0[:])
                    nc.vector.tensor_copy(t1[:], p1[:])
                    nc.vector.tensor_copy(t2[:], p2[:])
                    for hi, half in enumerate((0, F)):
                        op = ps.tile([hid, P], f32, tag=f"op{half}")
                        nc.tensor.matmul(
                            op[:], lhsT=w_sb[half : half + F, 0:hid],
                            rhs=t0[half : half + F, :], start=True, stop=False,
                        )
                        nc.tensor.matmul(
                            op[:], lhsT=w_sb[half : half + F, hid : 2 * hid],
                            rhs=t1[half : half + F, :], start=False, stop=False,
                        )
                        nc.tensor.matmul(
                            op[:], lhsT=w_sb[half : half + F, 2 * hid : 3 * hid],
                            rhs=t2[half : half + F, :], start=False, stop=True,
                        )
                        nc.scalar.activation(
                            oTs[0:hid, :], op[:], act, bias=b_sb[:, 0:1], scale=1.0
                        )
                        ph = ps.tile([P, hid], f32, tag=f"ph{half}")
                        nc.tensor.transpose(
                            out=ph[:], in_=oTs[0:hid, :], identity=ident[0:hid, 0:hid]
                        )
                        nc.vector.tensor_copy(
                            out_sb[:, ds(k * (2 * hid) + hi * hid, hid)], ph[:]
                        )

            # ---- layer 1 ----
            bscale(pub_sb, x_sb, dinv_sb)       # xtil = dinv*x
            publish(0)
            prop(0)                              # u_sb = u1
            bscale(tx1, u_sb, ndinv)             # Tx1 = -dinv*u1
            bscale(pub_sb, u_sb, ndinv2)         # Ttil1 = dinv*Tx1
            publish(1)
            prop(1)                              # u_sb = u2
            bscale(tx2, u_sb, n2dinv)            # -2dinv*u2
            nc.vector.tensor_tensor(
                out=tx2[:], in0=tx2[:], in1=x_sb[:], op=mybir.AluOpType.subtract
            )                                    # Tx2 = -2dinv*u2 - Tx0
            layer(x_sb, tx1, tx2, w1_sb, b1_sb, HID, h_sb,
                  mybir.ActivationFunctionType.Relu)

            # ---- layer 2 ----
            bscale(pub_sb, h_sb, dinv_sb)        # htil
            publish(2)
            prop(2)                              # u_sb = u3
            bscale(tx1, u_sb, ndinv)             # Tx1'
            bscale(pub_sb, u_sb, ndinv2)         # Ttil1'
            publish(3)
            prop(3)                              # u_sb = u4
            bscale(tx2, u_sb, n2dinv)
            nc.vector.tensor_tensor(
                out=tx2[:], in0=tx2[:], in1=h_sb[:], op=mybir.AluOpType.subtract
            )                                    # Tx2'
            layer(h_sb, tx1, tx2, w2_sb, b2_sb, C, o_sb,
                  mybir.ActivationFunctionType.Identity)

            # ---- stable log_softmax over C cols of each block ----
            ov = o_sb[:].rearrange("p (b c) -> p b c", c=C)
            mx = sb.tile([P, BLOCKS], f32)
            nc.vector.tensor_reduce(
                out=mx[:].unsqueeze(2), in_=ov,
                op=mybir.AluOpType.max, axis=mybir.AxisListType.X,
            )
            nc.vector.tensor_tensor(
                out=ov, in0=ov,
                in1=mx[:].unsqueeze(2).to_broadcast([P, BLOCKS, C]),
                op=mybir.AluOpType.subtract,
            )                                    # o_sb = shifted, in place
            e_sb = sb.tile([P, BLOCKS * C], f32)
            nc.scalar.activation(
                e_sb[:], o_sb[:], mybir.ActivationFunctionType.Exp
            )
            ssum = sb.tile([P, BLOCKS], f32)
            nc.vector.tensor_reduce(
                out=ssum[:].unsqueeze(2),
                in_=e_sb[:].rearrange("p (b c) -> p b c", c=C),
                op=mybir.AluOpType.add, axis=mybir.AxisListType.X,
            )
            lns = sb.tile([P, BLOCKS], f32)
            nc.scalar.activation(lns[:], ssum[:], mybir.ActivationFunctionType.Ln)
            nc.vector.tensor_tensor(
                out=ov, in0=ov,
                in1=lns[:].unsqueeze(2).to_broadcast([P, BLOCKS, C]),
                op=mybir.AluOpType.subtract,
            )                                    # o_sb = log_softmax, in place
            o16 = sb.tile([P, BLOCKS * C], f16)
            nc.vector.tensor_copy(o16[:], o_sb[:])
            nc.sync.dma_start(o_out.ap(), o16[:])

    nc.finalize()
    _cap_waits(nc)
    return nc


def _compile(nc, in_shapes, mesh):
    """AOT-compile the bass module for 8-core shard_map execution.

    Returns (compiled, in_order) where in_order lists input names (incl.
    donated output-zero buffers last) in executable argument order."""
    import jax
    from jax.experimental.shard_map import shard_map
    from jax.sharding import NamedSharding, PartitionSpec

    b2j.install_neuronx_cc_hook()
    partition_name = nc.partition_id_tensor.name if nc.partition_id_tensor else None
    in_names, out_names, out_avals = [], [], []
    for alloc in nc.m.functions[0].allocations:
        if not isinstance(alloc, mybir.MemoryLocationSet):
            continue
        name = alloc.memorylocations[0].name
        if alloc.kind == "ExternalInput":
            if name != partition_name:
                in_names.append(name)
        elif alloc.kind == "ExternalOutput":
            out_names.append(name)
            out_avals.append(
                jax.core.ShapedArray(
                    tuple(alloc.tensor_shape), mybir.dt.np(alloc.dtype)
                )
            )
    n_in, n_out = len(in_names), len(out_names)
    all_in = list(in_names) + list(out_names)
    if partition_name is not None:
        all_in.append(partition_name)

    def _body(*args):
        operands = list(args)
        if partition_name is not None:
            operands.append(b2j.partition_id_tensor())
        outs = b2j._bass_exec_p.bind(
            *operands,
            out_avals=tuple(out_avals),
            in_names=tuple(all_in),
            out_names=tuple(out_names),
            lowering_input_output_aliases=(),
            sim_require_finite=True,
            sim_require_nnan=True,
            nc=nc,
        )
        return tuple(outs)

    spec = PartitionSpec("core")
    jitted = jax.jit(
        shard_map(
            _body,
            mesh=mesh,
            in_specs=(spec,) * (n_in + n_out),
            out_specs=(spec,) * n_out,
            check_rep=False,
        ),
        donate_argnums=tuple(range(n_in, n_in + n_out)),
        keep_unused=True,
    )
    shd = NamedSharding(mesh, spec)
    avals = []
    for name in in_names:
        shape, dtype = in_shapes[name]
        avals.append(
            jax.ShapeDtypeStruct((NCORES * shape[0], *shape[1:]), dtype, sharding=shd)
        )
    for av in out_avals:
        avals.append(
            jax.ShapeDtypeStruct(
                (NCORES * av.shape[0], *av.shape[1:]), av.dtype, sharding=shd
            )
        )
    compiled = jitted.lower(*avals).compile()
    return compiled, in_names + out_names


def kernel(x, edge_index, W1, b1, W2, b2):
    x = np.asarray(x, np.float32)
    edge_index = np.asarray(edge_index, np.int32)
    W1 = np.asarray(W1, np.float32)
    b1 = np.asarray(b1, np.float32)
    W2 = np.asarray(W2, np.float32)
    b2 = np.asarray(b2, np.float32)

    state = {}

    def _warm():
        import jax

        state["devs"] = jax.devices()

    tw = threading.Thread(target=_warm, daemon=True)
    tw.start()

    (deg, dinv, slot_node, gid, slot_of, classes, colstart, tot_cols) = _prep_meta(
        edge_index
    )

    def _fill():
        idx, xb, dinvb = _prep_fill(
            x, edge_index, deg, dinv, slot_node, gid, slot_of, colstart, tot_cols
        )
        tw.join()
        import jax
        from jax.sharding import Mesh, NamedSharding, PartitionSpec

        mesh = Mesh(np.asarray(state["devs"][:NCORES]), ("core",))
        shd = NamedSharding(mesh, PartitionSpec("core"))
        arrs = {
            "x_in": xb.reshape(NCORES * P, BLOCKS * F),
            "dinv_in": dinvb.reshape(NCORES * P, BLOCKS),
            "idx_in": idx.reshape(NCORES * P, tot_cols),
            "w1_in": np.tile(W1, (NCORES, 1, 1)),
            "b1_in": np.tile(b1.reshape(1, HID), (NCORES, 1)),
            "w2_in": np.tile(W2, (NCORES, 1, 1)),
            "b2_in": np.tile(b2.reshape(1, C), (NCORES, 1)),
            "o_out": np.zeros((NCORES * P, BLOCKS * C), np.float16),
        }
        handles = {k: jax.device_put(v, shd) for k, v in arrs.items()}
        jax.block_until_ready(list(handles.values()))
        state["handles"] = handles

    tf = threading.Thread(target=_fill, daemon=True)
    tf.start()

    nc = _build(classes, tot_cols)

    tw.join()
    import jax
    from jax.sharding import Mesh

    mesh = Mesh(np.asarray(state["devs"][:NCORES]), ("core",))
    in_shapes = {
        "x_in": ((P, BLOCKS * F), np.float16),
        "dinv_in": ((P, BLOCKS), np.float32),
        "idx_in": ((P, tot_cols), np.int32),
        "w1_in": ((3, F, HID), np.float32),
        "b1_in": ((1, HID), np.float32),
        "w2_in": ((3, HID, C), np.float32),
        "b2_in": ((1, C), np.float32),
    }
    compiled, order = _compile(nc, in_shapes, mesh)

    tf.join()
    handles = state["handles"]
    out_arrs = compiled(*[handles[n] for n in order])
    ob = np.asarray(out_arrs[0]).astype(np.float32).reshape(NCORES, P, BLOCKS, C)
    rows = ob.transpose(0, 2, 1, 3).reshape(NCORES, SLOTS, C)
    out = np.empty((N, C), np.float32)
    out[slot_node] = rows[:, :NPC]
    return out
